# revision 30
# baseline (speedup 1.0000x reference)
"""Multi-head attention (B=8, N=1024, C=768, 12 heads) on 8 Trainium2 cores.

Strategy: data-parallel over batch — one batch element per NeuronCore, no
collectives. Per core everything stays on-chip:

  1. qkv projection in two orientations:
       - Q^T/K^T tiles [d3, tok]  (lhsT = w_qkv slices, rhs = x^T)
       - V tiles      [tok, d]    (lhsT = x^T slices,  rhs = w_qkv V-columns)
     Only the head-0/1 Q and K chains run up front; the remaining ten chains
     are spread one-per-head through the attention stream (split into
     512-token halves for the last pair so head 10's operands land a full
     head early) so the PE fills its exp-wait gaps.
  2. attention per head, software-pipelined: per kt-slot the PE-queue order
     is [fillers, PV(kt-2), ST(kt)] so the in-order PE queue never
     head-of-line blocks on ST's PSUM-bank wait. Scores S^T[k, q] = K_h Q_h^T
     with zero-padded K variants so everything stays in 128x128 PE tiling
     mode, exp on ScalarE over [128, 1024] tiles (scale=1/8 folded in), then
     PV with a ones-column at position 0 of the V stationary so PSUM row 0 is
     the softmax denominator. Normalization: DVE reciprocal -> GpSimd
     partition-broadcast -> GpSimd multiply (keeps DVE free for evacuations).
     The last drain slot runs qh0's final PVs and norm before qh1's
     ("eager norm") so the normalize chain overlaps the remaining PV stream.
  3. proj: ct 0..4 partials accumulate during heads 10/11 in their own PSUM
     ring; the ct=5 tail runs per qh-half right after the corresponding
     head-11 norm, evacuating through DVE (qh0) and ScalarE-copy+DVE (qh1).

PSUM plan (8 banks): scores double-buffer 2x[128,1024] (4 banks) is never
shared with anything else; attention-out accumulators 2x[128,512] (2 banks);
a general 2x[128,512] ring (2 banks) serves warmup, qkv-chain psums, V-chain
psums, proj partials and the proj tail in program order.

Input DMAs are priority-ordered on one queue so bytes land in consumption
order: x^T, w_qkv Q/K columns for head-pair 0, V columns, remaining Q/K
columns, w_proj. Biases ride a second queue. Dummy matmuls ride out the
input-DMA prologue to keep the PE activity monitor at full clock.

Numerics: all matmul operands fp16, fp32 PSUM accumulation and fp32 softmax:
~7e-4 max rel err vs the fp32 reference.
"""

import numpy as np

import concourse.bass as bass
import concourse.tile as tile
import concourse.mybir as mybir
from concourse import bacc
from concourse.bass_utils import run_bass_kernel_spmd

dt = mybir.dt
AF = mybir.ActivationFunctionType
ALU = mybir.AluOpType

B = 8
C = 768
N = 1024          # tokens per batch element (32*32)
NH = 12           # heads
HD = 64           # head dim
C3 = 3 * C        # 2304
CT = C // 128     # 6 contraction tiles
TT = N // 128     # 8 token tiles
NQH = 2           # q processed in halves of 512 where PSUM-bank-bound
QHW = N // NQH    # 512
SCALE = HD ** -0.5
N_WARMUP_MM = 26  # dummy matmuls riding out the input-DMA prologue


def _build_nc():
    nc = bacc.Bacc(None, target_bir_lowering=False)

    # All big inputs arrive HOST-PRE-TILED in SBUF layout [128, CT, w] so
    # every DMA is a fully contiguous streaming copy (multi-KB per partition
    # line) instead of a strided gather of 256B..2KB chunks — the early-DMA
    # phase is bandwidth-critical. w_qkv is also column-REORDERED host-side
    # into consumption-priority regions: Q/K for head-pair 0 first, then V
    # columns, then the remaining Q/K pairs.
    xt_ext = nc.dram_tensor("xt", [128, CT, N], dt.float16, kind="ExternalInput")
    wq_r0_ext = nc.dram_tensor("wq_r0", [128, CT, 128], dt.float16, kind="ExternalInput")
    wq_r6_ext = nc.dram_tensor("wq_r6", [128, CT, 128], dt.float16, kind="ExternalInput")
    wq_v_ext = nc.dram_tensor("wq_v", [128, CT, C], dt.float16, kind="ExternalInput")
    wq_q_ext = nc.dram_tensor("wq_q", [128, CT, 640], dt.float16, kind="ExternalInput")
    wq_k_ext = nc.dram_tensor("wq_k", [128, CT, 640], dt.float16, kind="ExternalInput")
    bqk_ext = nc.dram_tensor("b_qkt", [128, 2 * C // 128], dt.float32, kind="ExternalInput")
    bv_ext = nc.dram_tensor("b_v", [1, C], dt.float32, kind="ExternalInput")
    wp_ext = nc.dram_tensor("w_proj", [128, CT, C], dt.float16, kind="ExternalInput")
    bp_ext = nc.dram_tensor("b_proj", [128, C // 128], dt.float32, kind="ExternalInput")
    y_ext = nc.dram_tensor("y", [C, N], dt.float16, kind="ExternalOutput")

    with (
        tile.TileContext(nc) as tc,
        tc.tile_pool(name="persist", bufs=1) as pp,
        tc.tile_pool(name="qkrot", bufs=3) as qkrot,
        tc.tile_pool(name="ps", bufs=2, space="PSUM") as ps,
        tc.tile_pool(name="att_sb", bufs=12) as att_sb,
        tc.tile_pool(name="att_small", bufs=3) as att_small,
        tc.tile_pool(name="y_sb", bufs=4) as y_pool,
    ):
        def ps_st(name):
            return ps.tile([128, N], dt.float32, tag="st", bufs=2, name=name)

        def ps_pov(name):
            return ps.tile([128, QHW], dt.float32, tag="pov", bufs=2, name=name)

        def ps_qkp(name):
            return ps.tile([128, QHW], dt.float32, tag="qkp", bufs=2, name=name)

        # ---- constants / biases ----
        ones_f32 = pp.tile([128, NH, 1], dt.float32, tag="ones_f32")
        nc.vector.memset(ones_f32[:], 1.0)
        bqk_sb = pp.tile([128, 2 * C // 128], dt.float32, tag="bqk")
        bv_sb = pp.tile([128, C], dt.float32, tag="bv")
        bp_sb = pp.tile([128, C // 128], dt.float32, tag="bp")

        dummy_sb = pp.tile([128, QHW], dt.float16, tag="dummy_sb")
        nc.vector.memset(dummy_sb[:].bitcast(dt.float32), 0.0)

        # ---- input loads: one priority-ordered queue for the big tensors
        # (descriptors drain in order). Every copy is contiguous because the
        # host pre-tiled the arrays. Biases ride a second queue (tiny).
        xt_all = pp.tile([128, CT, N], dt.float16, tag="xt_all")
        wq_r0_t = pp.tile([128, CT, 128], dt.float16, tag="wq_r0")
        wq_r6_t = pp.tile([128, CT, 128], dt.float16, tag="wq_r6")
        wq_v_t = pp.tile([128, CT, C], dt.float16, tag="wq_v")
        wq_q_t = pp.tile([128, CT, 640], dt.float16, tag="wq_q")
        wq_k_t = pp.tile([128, CT, 640], dt.float16, tag="wq_k")
        wp_all = pp.tile([128, CT, C], dt.float16, tag="wp_all")

        # x^T lands per-ct so the first qk-chain matmuls start as soon as the
        # first 256KB arrives (the DMA engines ramp slowly early on); V lands
        # in ct-pair blocks consumed in order by att0's V-chain fillers
        nc.sync.dma_start(out=wq_r0_t[:], in_=wq_r0_ext[:, :, :])
        nc.sync.dma_start(out=wq_r6_t[:], in_=wq_r6_ext[:, :, :])
        for i in range(CT):
            nc.sync.dma_start(out=xt_all[:, i, :], in_=xt_ext[:, i, :])
        for i in range(3):
            nc.sync.dma_start(
                out=wq_v_t[:, 2 * i:2 * i + 2, :], in_=wq_v_ext[:, 2 * i:2 * i + 2, :]
            )
        nc.sync.dma_start(out=wq_q_t[:], in_=wq_q_ext[:, :, :])
        nc.sync.dma_start(out=wq_k_t[:], in_=wq_k_ext[:, :, :])
        nc.sync.dma_start(out=wp_all[:], in_=wp_ext[:, :, :])
        nc.gpsimd.dma_start(out=bqk_sb[:], in_=bqk_ext[:, :])
        nc.gpsimd.dma_start(out=bv_sb[:], in_=bv_ext[0:1, :].to_broadcast((128, C)))
        nc.gpsimd.dma_start(out=bp_sb[:], in_=bp_ext[:, :])

        xt_sb = [xt_all[:, i, :] for i in range(CT)]
        wp_sb = [wp_all[:, i, :] for i in range(CT)]

        def wq_stat(d3, ct):
            """Stationary [128, 128] for Q/K d3-tile from the reordered regions."""
            if d3 == 0:
                return wq_r0_t[:, ct, :]
            if d3 == 6:
                return wq_r6_t[:, ct, :]
            if d3 < 6:
                return wq_q_t[:, ct, 128 * (d3 - 1):128 * d3]
            return wq_k_t[:, ct, 128 * (d3 - 7):128 * (d3 - 6)]

        # attn_out^T: 6 persistent tiles of [128, N]
        aT = [pp.tile([128, N], dt.float16, name=f"aT{i}", tag=f"aT{i}") for i in range(CT)]
        # V per token-tile [128, NH, 128] fp16: col 0 = ones (softmax
        # denominator row), cols 1:64 zero, cols 64:128 = V for that head.
        v_sb = [pp.tile([128, NH, 128], dt.float16, name=f"v{i}", tag=f"v{i}") for i in range(TT)]
        # proj ct0..4 partials (+ bias), fp16
        yacc = [pp.tile([128, N], dt.float16, name=f"yacc{i}", tag=f"yacc{i}")
                for i in range(CT)]

        # rotating Q^T/K^T tiles, keyed by d3-tile index
        qkT = {}

        def qk_half_thunks(d3, qh):
            """One qh-half of a Q^T/K^T projection chain: 6 matmuls into a
            [128,512] psum + bias evac into the rotating qkT tile. K tiles
            (d3 >= 6) are written as TWO zero-padded variants (even head in
            rows 0:64 / zeros, odd head in rows 64:128 / zeros) so the score
            matmuls run with K=128 contraction against the packed Q — keeps
            the PE in 128x128 tiling mode for the whole kernel."""
            thunks = []
            if qh == 0:
                thunks.extend(qk_make_tiles(d3))

            pss = ps_qkp(f"ps_qk{d3}_{qh}")

            def mk_mm(ct):
                def run():
                    nc.tensor.matmul(
                        out=pss[:],
                        lhsT=wq_stat(d3, ct),
                        rhs=xt_sb[ct][:, QHW * qh:QHW * (qh + 1)],
                        start=(ct == 0), stop=(ct == CT - 1),
                    )
                return run

            for ct in range(CT):
                thunks.append(mk_mm(ct))
            thunks.append(lambda: qk_bias(d3, qh, pss))
            return thunks

        def qk_make_tiles(d3):
            """Create the rotating qkT tile(s) for d3; returns setup thunks."""
            if d3 < 6:
                qkT[d3] = qkrot.tile([128, N], dt.float16, tag="qv", name=f"qkT{d3}")
                return []
            kz_e = qkrot.tile([128, N], dt.float16, tag="kve", name=f"kzE{d3}")
            kz_o = qkrot.tile([128, N], dt.float16, tag="kvo", name=f"kzO{d3}")
            qkT[d3] = (kz_e, kz_o)

            def zero():
                nc.vector.memset(kz_e[64:128, :].bitcast(dt.float32), 0.0)
                nc.vector.memset(kz_o[0:64, :].bitcast(dt.float32), 0.0)
            return [zero]

        def qk_bias(d3, qh, pss):
            if d3 < 6:
                nc.vector.tensor_scalar(
                    out=qkT[d3][:, QHW * qh:QHW * (qh + 1)], in0=pss[:],
                    scalar1=bqk_sb[:, d3:d3 + 1], scalar2=None, op0=ALU.add,
                )
            else:
                kz_e, kz_o = qkT[d3]
                nc.vector.tensor_scalar(
                    out=kz_e[0:64, QHW * qh:QHW * (qh + 1)],
                    in0=pss[0:64, :],
                    scalar1=bqk_sb[0:64, d3:d3 + 1], scalar2=None, op0=ALU.add,
                )
                nc.vector.tensor_scalar(
                    out=kz_o[64:128, QHW * qh:QHW * (qh + 1)],
                    in0=pss[64:128, :],
                    scalar1=bqk_sb[64:128, d3:d3 + 1], scalar2=None, op0=ALU.add,
                )

        def qk_chain_thunks(d3):
            return qk_half_thunks(d3, 0) + qk_half_thunks(d3, 1)

        def att_head(h, fillers=(), last_slot_fillers=()):
            q_tile = qkT[h // 2]
            kz = qkT[6 + h // 2][h % 2]
            po = 64 * (h % 2)
            fillers = list(fillers)
            # Software-pipelined head: per kt-slot emit fillers, then
            # PV(kt-2), then ST(kt)+exp(kt). ST waits on exp(kt-2)'s PSUM
            # bank; emitting the independent work first keeps the in-order
            # PE queue busy through that wait.
            PIPE = 2
            ess = []
            povs = [ps_pov(f"pov{h}_{qh}") for qh in range(NQH)]
            fi = 0
            # 2 kt per slot: batching two kt of STs (and two kt of PVs) per
            # slot halves the psum-ring handoffs.
            NSLOT = TT // 2 + PIPE

            def pv_one(kt, qh):
                nc.tensor.matmul(
                    out=povs[qh][:],
                    lhsT=v_sb[kt][:, h, :],
                    rhs=ess[kt][:, QHW * qh:QHW * (qh + 1)],
                    start=(kt == 0), stop=(kt == TT - 1),
                )

            def pv_pair(kt):
                for qh in range(NQH):
                    pv_one(kt, qh)

            def norm_one(qh):
                # normalize rows 64:128 by reciprocal of denominator row 0:
                # DVE reciprocal, GpSimd broadcast, DVE multiply (GpSimd
                # cannot read PSUM)
                r_sb = att_small.tile([1, QHW], dt.float32, tag="r")
                nc.vector.reciprocal_approx_fast(out=r_sb[:], in_=povs[qh][0:1, :])
                rb_sb = att_small.tile([HD, QHW], dt.float32, tag="rb")
                nc.gpsimd.partition_broadcast(rb_sb[:], r_sb[:])
                nc.vector.tensor_tensor(
                    out=aT[h // 2][po:po + HD, QHW * qh:QHW * (qh + 1)],
                    in0=povs[qh][64:128, :],
                    in1=rb_sb[:],
                    op=ALU.mult,
                )

            for sl in range(NSLOT):
                # fillers first: anything a PV may consume (e.g. att0's
                # V tiles) must be emitted before the PV that reads it
                want = min(len(fillers), -(-(sl * len(fillers)) // (NSLOT - 2)))
                while fi < want:
                    fillers[fi]()
                    fi += 1
                if sl >= PIPE:
                    k0 = 2 * (sl - PIPE)
                    if sl == NSLOT - 1:
                        # eager norm: finish qh0's accumulation (any kt order
                        # is fine — it's a sum) and start its normalize chain
                        # while qh1's PVs (and any held-back fillers) stream
                        pv_one(k0, 0)
                        pv_one(k0 + 1, 0)
                        norm_one(0)
                        for th in last_slot_fillers:
                            th()
                        pv_one(k0, 1)
                        pv_one(k0 + 1, 1)
                        norm_one(1)
                    else:
                        pv_pair(k0)
                        pv_pair(k0 + 1)
                if sl < TT // 2:
                    pair = []
                    for kt in (2 * sl, 2 * sl + 1):
                        pss = ps_st(f"pss{h}_{kt}")
                        for qh in range(NQH):
                            nc.tensor.matmul(
                                out=pss[:, QHW * qh:QHW * (qh + 1)],
                                lhsT=kz[:, 128 * kt:128 * (kt + 1)],
                                rhs=q_tile[:, QHW * qh:QHW * (qh + 1)],
                                start=True, stop=True,
                            )
                        pair.append(pss)
                    for kt in (2 * sl, 2 * sl + 1):
                        es = att_sb.tile([128, N], dt.float16, tag="es", name=f"es{h}_{kt}")
                        nc.scalar.activation(
                            out=es[:], in_=pair[kt - 2 * sl][:], func=AF.Exp, scale=SCALE
                        )
                        ess.append(es)
            while fi < len(fillers):
                fillers[fi]()
                fi += 1

        def att_head_qh(h, qh, fillers=(), filler_start=0):
            """One qh-phase of a head (used for the final head so the proj
            tail for the finished half can stream as the other half's
            fillers). STs write a [kt_even | kt_odd] pair tile for this qh
            only, so exp stays one [128, N] ACT per slot — same ScalarE cost
            as the full-head path. `filler_start` delays filler emission to
            slot >= filler_start so fillers gated on this head's earlier
            phase don't head-of-line block the PE queue."""
            q_tile = qkT[h // 2]
            kz = qkT[6 + h // 2][h % 2]
            po = 64 * (h % 2)
            fillers = list(fillers)
            PIPE = 2
            NSLOT = TT // 2 + PIPE
            pov = ps_pov(f"pov{h}q{qh}")
            espairs = []
            fi = 0

            def pv_one(kt):
                nc.tensor.matmul(
                    out=pov[:],
                    lhsT=v_sb[kt][:, h, :],
                    rhs=espairs[kt // 2][:, QHW * (kt % 2):QHW * (kt % 2 + 1)],
                    start=(kt == 0), stop=(kt == TT - 1),
                )

            for sl in range(NSLOT):
                eff = max(0, sl - filler_start)
                span = NSLOT - 2 - filler_start
                want = min(len(fillers), -(-(eff * len(fillers)) // span))
                while fi < want:
                    fillers[fi]()
                    fi += 1
                if sl >= PIPE:
                    k0 = 2 * (sl - PIPE)
                    pv_one(k0)
                    pv_one(k0 + 1)
                if sl < TT // 2:
                    pss = ps_st(f"pss{h}q{qh}_{sl}")
                    for kt in (2 * sl, 2 * sl + 1):
                        nc.tensor.matmul(
                            out=pss[:, QHW * (kt % 2):QHW * (kt % 2 + 1)],
                            lhsT=kz[:, 128 * kt:128 * (kt + 1)],
                            rhs=q_tile[:, QHW * qh:QHW * (qh + 1)],
                            start=True, stop=True,
                        )
                    es = att_sb.tile([128, N], dt.float16, tag="es",
                                     name=f"es{h}q{qh}_{sl}")
                    nc.scalar.activation(out=es[:], in_=pss[:], func=AF.Exp, scale=SCALE)
                    espairs.append(es)
            while fi < len(fillers):
                fillers[fi]()
                fi += 1
            # this phase's normalize: the next phase's (or the tail's) PE
            # stream covers the reciprocal->broadcast->multiply latency
            r_sb = att_small.tile([1, QHW], dt.float32, tag="r")
            nc.vector.reciprocal_approx_fast(out=r_sb[:], in_=pov[0:1, :])
            rb_sb = att_small.tile([HD, QHW], dt.float32, tag="rb")
            nc.gpsimd.partition_broadcast(rb_sb[:], r_sb[:])
            nc.vector.tensor_tensor(
                out=aT[h // 2][po:po + HD, QHW * qh:QHW * (qh + 1)],
                in0=pov[64:128, :],
                in1=rb_sb[:],
                op=ALU.mult,
            )

        # HAM warm-up: dummy matmuls with no input dependencies, all into ONE
        # psum tile — same-engine WAW ordering is implicit, so they run
        # back-to-back with no ring-semaphore waits
        pwarm = ps_qkp("pwarm")
        for i in range(N_WARMUP_MM):
            nc.tensor.matmul(
                out=pwarm[:], lhsT=dummy_sb[:, 0:128], rhs=dummy_sb[:],
                start=True, stop=True, skip_group_check=True,
            )

        # prologue chains for head-pair 0, ct-grouped: all four psum chains
        # run their ct 0..2 matmuls as soon as x^T's first half lands, while
        # the second half is still in flight
        for th in qk_make_tiles(0) + qk_make_tiles(6):
            th()
        pro_ps = {
            (0, 0): ps_qkp("ps_pro0_0"), (0, 1): ps_qkp("ps_pro0_1"),
            (6, 0): ps_pov("ps_pro6_0"), (6, 1): ps_pov("ps_pro6_1"),
        }
        for ct in range(CT):
            for d3 in (0, 6):
                for qh in range(NQH):
                    nc.tensor.matmul(
                        out=pro_ps[(d3, qh)][:],
                        lhsT=wq_stat(d3, ct),
                        rhs=xt_sb[ct][:, QHW * qh:QHW * (qh + 1)],
                        start=(ct == 0), stop=(ct == CT - 1),
                    )
        for d3 in (0, 6):
            for qh in range(NQH):
                qk_bias(d3, qh, pro_ps[(d3, qh)])

        # V part of the qkv projection, as thunk lists. Split per 512/256
        # column half so each psum lives in one [128,512] ring slot.
        def v_chain_thunks(tt):
            thunks = []

            def zero():
                nc.vector.memset(v_sb[tt][:].bitcast(dt.float32), 0.0)
            thunks.append(zero)

            psA = ps_qkp(f"ps_v{tt}_a")
            psB = ps_qkp(f"ps_v{tt}_b")

            def mk_mm(pst, w, c0, ct):
                def run():
                    nc.tensor.matmul(
                        out=pst[:, 0:w],
                        lhsT=xt_sb[ct][:, 128 * tt:128 * (tt + 1)],
                        rhs=wq_v_t[:, ct, c0:c0 + w],
                        start=(ct == 0), stop=(ct == CT - 1),
                    )
                return run

            for ct in range(CT):
                thunks.append(mk_mm(psA, 512, 0, ct))

            def finishA():
                nc.vector.tensor_tensor(
                    out=v_sb[tt][:, 0:8, 64:128],
                    in0=psA[:].rearrange("p (h d) -> p h d", h=8),
                    in1=bv_sb[:, 0:512].rearrange("p (h d) -> p h d", h=8),
                    op=ALU.add,
                )
            thunks.append(finishA)

            for ct in range(CT):
                thunks.append(mk_mm(psB, 256, 512, ct))

            def finishB():
                nc.vector.tensor_tensor(
                    out=v_sb[tt][:, 8:12, 64:128],
                    in0=psB[:, 0:256].rearrange("p (h d) -> p h d", h=4),
                    in1=bv_sb[:, 512:768].rearrange("p (h d) -> p h d", h=4),
                    op=ALU.add,
                )
                nc.vector.tensor_copy(out=v_sb[tt][:, :, 0:1], in_=ones_f32[:])
            thunks.append(finishB)
            return thunks

        # proj partials: per co, the ct 0..4 partial (heads 0..9's channels)
        # runs as one thunk per qh-half during heads 10/11 — 5 matmuls into a
        # [128,512] ring slot, then a bias-folding evac to fp16 yacc.
        def proj_partial_thunks(co):
            thunks = []
            for qh in range(NQH):
                def run(qh=qh):
                    psq = ps_qkp(f"ps_yp{co}_{qh}")
                    for ct in range(CT - 1):
                        nc.tensor.matmul(
                            out=psq[:],
                            lhsT=wp_sb[ct][:, 128 * co:128 * (co + 1)],
                            rhs=aT[ct][:, QHW * qh:QHW * (qh + 1)],
                            start=(ct == 0), stop=(ct == CT - 2),
                        )
                    nc.vector.tensor_scalar(
                        out=yacc[co][:, QHW * qh:QHW * (qh + 1)], in0=psq[:],
                        scalar1=bp_sb[:, co:co + 1], scalar2=None, op0=ALU.add,
                    )
                thunks.append(run)
            return thunks

        # attention heads 0..11 with remaining qkv work sprinkled into
        # each head's PE stream: att0 carries ALL V tiles; heads 1..7 carry
        # the Q/K chains for pairs 1..4 plus pair 5's first halves pushed to
        # heads 8/9 as qh-split halves so head 10's operands are ready a full
        # head early; heads 10/11 carry the proj ct0..4 partials.
        filler_plan = {
            0: lambda: [t for tt in range(TT) for t in v_chain_thunks(tt)],
            1: lambda: qk_chain_thunks(1) + qk_chain_thunks(7),
            2: lambda: qk_chain_thunks(2),
            3: lambda: qk_chain_thunks(8),
            4: lambda: qk_chain_thunks(3),
            5: lambda: qk_chain_thunks(9),
            6: lambda: qk_chain_thunks(4),
            7: lambda: qk_chain_thunks(10),
            8: lambda: qk_half_thunks(5, 0) + qk_half_thunks(11, 0),
            9: lambda: qk_half_thunks(5, 1) + qk_half_thunks(11, 1),
            10: lambda: [t for co in (0, 1, 2) for t in proj_partial_thunks(co)],
        }

        # ---- proj tail rungs: ct=5 contribution for one (co, qh) plus the
        # yacc add and the y DMA. qh0 rungs stream as fillers of head 11's
        # second attention phase; qh1 rungs drain after it. Evac engines
        # alternate (DVE psum-add vs ScalarE copy + DVE fp16 2x add) so the
        # final ladder isn't serialized on one engine.
        def tail_rung(co, qh):
            def run():
                pst = ps_qkp(f"ps_y{co}_{qh}")
                nc.tensor.matmul(
                    out=pst[:],
                    lhsT=wp_sb[CT - 1][:, 128 * co:128 * (co + 1)],
                    rhs=aT[CT - 1][:, QHW * qh:QHW * (qh + 1)],
                    start=True, stop=True,
                )
                y_half = y_pool.tile([128, QHW], dt.float16, tag="y")
                nc.vector.tensor_tensor(
                    out=y_half[:], in0=pst[:],
                    in1=yacc[co][:, QHW * qh:QHW * (qh + 1)], op=ALU.add,
                )
                nc.sync.dma_start(
                    out=y_ext[128 * co:128 * (co + 1), QHW * qh:QHW * (qh + 1)],
                    in_=y_half[:],
                )
            return run

        def tail_wide_qh1():
            # final drain: by now the score double-buffer banks are free, so
            # all six ct=5 matmuls get their own psum bank and run
            # back-to-back; evacs alternate DVE-direct / ScalarE-copy+DVE so
            # the ladder drains on two engines; all y DMAs issue on sync.
            psts = []
            for pair in range(2):
                stw = ps_st(f"ps_yw{pair}")
                psts += [stw[:, 0:QHW], stw[:, QHW:N]]
            psts += [ps_qkp("ps_yw4"), ps_qkp("ps_yw5")]
            for co in range(CT):
                nc.tensor.matmul(
                    out=psts[co][:],
                    lhsT=wp_sb[CT - 1][:, 128 * co:128 * (co + 1)],
                    rhs=aT[CT - 1][:, QHW:N],
                    start=True, stop=True,
                )
            for co in range(CT):
                y_half = y_pool.tile([128, QHW], dt.float16, tag="y")
                if co % 2 == 0:
                    nc.vector.tensor_tensor(
                        out=y_half[:], in0=psts[co][:],
                        in1=yacc[co][:, QHW:N], op=ALU.add,
                    )
                else:
                    t16 = y_pool.tile([128, QHW], dt.float16, tag="yt", bufs=2)
                    nc.scalar.activation(out=t16[:], in_=psts[co][:], func=AF.Copy)
                    nc.vector.tensor_tensor(
                        out=y_half[:], in0=t16[:],
                        in1=yacc[co][:, QHW:N], op=ALU.add,
                    )
                # alternate issue queues so six ~0.6us DMA issues don't
                # serialize on one engine at the very end
                (nc.sync if co % 2 == 0 else nc.gpsimd).dma_start(
                    out=y_ext[128 * co:128 * (co + 1), QHW:N],
                    in_=y_half[:],
                )

        for h in range(NH - 1):
            att_head(h, filler_plan[h]() if h in filler_plan else ())
        # head 11 runs as two sequential qh-phases: phase 0 carries the
        # remaining proj partials; phase 1 carries the qh0 proj-tail rungs
        # (gated on phase 0's norm — filler_start keeps them from
        # head-of-line blocking phase 1's first STs)
        att_head_qh(NH - 1, 0, [t for co in (3, 4, 5) for t in proj_partial_thunks(co)])
        att_head_qh(NH - 1, 1, [tail_rung(co, 0) for co in range(CT)], filler_start=2)
        tail_wide_qh1()

    nc.compile()
    return nc


_NC_CACHE = {}


def kernel(x, w_qkv, b_qkv, w_proj, b_proj, _trace=False):
    x = np.asarray(x, dtype=np.float32)
    w_qkv = np.asarray(w_qkv, dtype=np.float32)
    b_qkv = np.asarray(b_qkv, dtype=np.float32)
    w_proj = np.asarray(w_proj, dtype=np.float32)
    b_proj = np.asarray(b_proj, dtype=np.float32)

    if "nc" not in _NC_CACHE:
        _NC_CACHE["nc"] = _build_nc()
    nc = _NC_CACHE["nc"]

    # host-side prep (pure layout, no arithmetic). Big tensors are shipped
    # pre-tiled [128, CT, w] so every device DMA is a contiguous stream.
    def tiled(a2d):
        # [CT*128, w] -> [128, CT, w]
        ctn = a2d.shape[0] // 128
        return np.ascontiguousarray(
            a2d.reshape(ctn, 128, a2d.shape[1]).transpose(1, 0, 2)
        )

    # b_qkt: Q/K bias columns laid out per d3-tile: [128, 12]
    b_qkt = np.ascontiguousarray(b_qkv[:2 * C].reshape(2 * C // 128, 128).T)
    w_qkv_h = w_qkv.astype(np.float16)
    wq_r0 = tiled(w_qkv_h[:, 0:128])
    wq_r6 = tiled(w_qkv_h[:, 768:896])
    wq_v = tiled(w_qkv_h[:, 1536:2304])
    wq_q = tiled(w_qkv_h[:, 128:768])
    wq_k = tiled(w_qkv_h[:, 896:1536])
    w_proj_t = tiled(w_proj.astype(np.float16))
    b_v = np.ascontiguousarray(b_qkv[2 * C:].reshape(1, C))
    b_p = np.ascontiguousarray(b_proj.reshape(C // 128, 128).T)

    core_ids = list(range(B))
    in_maps = []
    for b in range(B):
        xt = tiled(x[b].reshape(N, C).T.astype(np.float16))  # [128, 6, N]
        in_maps.append({
            "xt": xt,
            "wq_r0": wq_r0,
            "wq_r6": wq_r6,
            "wq_v": wq_v,
            "wq_q": wq_q,
            "wq_k": wq_k,
            "b_qkt": b_qkt,
            "b_v": b_v,
            "w_proj": w_proj_t,
            "b_proj": b_p,
        })

    res = run_bass_kernel_spmd(nc, in_maps, core_ids, trace=_trace)
    if _trace:
        _NC_CACHE["last_result"] = res

    out = np.empty((B, 32, 32, C), dtype=np.float32)
    for b in range(B):
        out[b] = res.results[b]["y"].T.reshape(32, 32, C)
    return out


# revision 45
# speedup vs baseline: 1.2196x; 1.2196x over previous
"""Multi-head attention (B=8, N=1024, C=768, 12 heads) on 8 Trainium2 cores.

Strategy: data-parallel over batch — one batch element per NeuronCore, no
collectives. Per core everything stays on-chip:

  1. qkv projection in two orientations:
       - Q^T/K^T tiles [d3, tok]  (lhsT = w_qkv slices, rhs = x^T)
       - V tiles      [tok, d]    (lhsT = x^T slices,  rhs = w_qkv V-columns)
     The head-0/1 Q and K chains run up front, ct-grouped across all four
     psum chains so the ct 0..2 matmuls stream while x^T's later tiles are
     still in DMA flight. The remaining ten chains are spread one-per-head
     through the attention stream (split into 512-token halves for the last
     pair so head 10's operands land a full head early) so the PE fills its
     exp-wait gaps.
  2. attention per head, software-pipelined: per kt-slot the PE-queue order
     is [fillers, PV(kt-2), ST(kt)] so the in-order PE queue never
     head-of-line blocks on ST's PSUM-bank wait. Scores S^T[k, q] = K_h Q_h^T
     with zero-padded K variants so everything stays in 128x128 PE tiling
     mode, exp on ScalarE over [128, 1024] tiles (scale=1/8 folded in), then
     PV with a ones-column at position 0 of the V stationary so PSUM row 0 is
     the softmax denominator. Normalization: DVE reciprocal -> GpSimd
     partition-broadcast -> DVE multiply (GpSimd cannot read PSUM). Each
     head's last slot finishes qh0's accumulation and starts its normalize
     chain while qh1's PVs still stream ("eager norm").
  3. the final head runs as two sequential q-half PHASES: phase 0 (q 0:512)
     carries the last proj partials as fillers and norms its half at phase
     end; phase 1 (q 512:1024) carries the qh0 proj-tail rungs as fillers
     (each rung: ct=5 matmul + yacc add + y DMA), so 3/4 of the output is
     written while attention still streams. The co5 partial's qh1 matmuls
     are emitted between phase 1's last PVs and its norm so they cover the
     reciprocal->broadcast->multiply latency; the remaining qh1 tail runs
     "wide" — all six ct=5 matmuls into dedicated psum banks (the score
     double-buffer is free by then), back-to-back DVE evacs, y DMAs
     alternating between the idle sync/gpsimd queues.
  4. proj ct 0..4 partials accumulate during heads 10 / 11-phase-0 in the
     shared [128,512] psum ring, evacuating per q-half into fp16 yacc tiles
     with the bias folded in.

PSUM plan (8 banks): scores double-buffer 2x[128,1024] (4 banks, reused by
the wide tail at the very end); attention-out accumulators 2x[128,512]
(2 banks); a general 2x[128,512] ring (2 banks) serves warmup, qkv-chain
psums, V-chain psums, proj partials and tail rungs in program order.

Inputs arrive HOST-PRE-TILED in SBUF layout [128, ct, w] (and w_qkv
column-reordered into consumption-priority regions) so every input DMA is a
fully contiguous multi-KB-per-partition-line stream — the DMA engines ramp
slowly in the first ~10us and strided gathers run several times slower.
Priority order on one queue: Q/K pair-0 stationaries, x^T per-ct (the first
chain matmuls start on the first 256KB), V columns in ct-pair blocks,
remaining Q/K, w_proj; biases on a second queue. Dummy matmuls ride out the
input-DMA prologue to keep the PE activity monitor at full clock (idle gaps
also drop the PE to a half-speed p-state for ~3us, so gaps cost double).

Numerics: all matmul operands fp16, fp32 PSUM accumulation and fp32 softmax:
~7e-4 max rel err vs the fp32 reference. (fp8 was evaluated and rejected:
e4m3 scores/PV give 2.8-4.5e-2 error vs the 2e-2 gate.)
"""

import numpy as np

import concourse.bass as bass
import concourse.tile as tile
import concourse.mybir as mybir
from concourse import bacc
from concourse.bass_utils import run_bass_kernel_spmd

dt = mybir.dt
AF = mybir.ActivationFunctionType
ALU = mybir.AluOpType

B = 8
C = 768
N = 1024          # tokens per batch element (32*32)
NH = 12           # heads
HD = 64           # head dim
C3 = 3 * C        # 2304
CT = C // 128     # 6 contraction tiles
TT = N // 128     # 8 token tiles
NQH = 2           # q processed in halves of 512 where PSUM-bank-bound
QHW = N // NQH    # 512
SCALE = HD ** -0.5
N_WARMUP_MM = 26  # dummy matmuls riding out the input-DMA prologue


def _build_nc():
    nc = bacc.Bacc(None, target_bir_lowering=False)

    # All big inputs arrive HOST-PRE-TILED in SBUF layout [128, CT, w] so
    # every DMA is a fully contiguous streaming copy (multi-KB per partition
    # line) instead of a strided gather of 256B..2KB chunks — the early-DMA
    # phase is bandwidth-critical. w_qkv is also column-REORDERED host-side
    # into consumption-priority regions: Q/K for head-pair 0 first, then V
    # columns, then the remaining Q/K pairs.
    xt_ext = nc.dram_tensor("xt", [128, CT, N], dt.float16, kind="ExternalInput")
    wq_r0_ext = nc.dram_tensor("wq_r0", [128, CT, 128], dt.float16, kind="ExternalInput")
    wq_r6_ext = nc.dram_tensor("wq_r6", [128, CT, 128], dt.float16, kind="ExternalInput")
    wq_v_ext = nc.dram_tensor("wq_v", [128, CT, C], dt.float16, kind="ExternalInput")
    wq_q_ext = nc.dram_tensor("wq_q", [128, CT, 640], dt.float16, kind="ExternalInput")
    wq_k_ext = nc.dram_tensor("wq_k", [128, CT, 640], dt.float16, kind="ExternalInput")
    bqk_ext = nc.dram_tensor("b_qkt", [128, 2 * C // 128], dt.float32, kind="ExternalInput")
    bv_ext = nc.dram_tensor("b_v", [1, C], dt.float32, kind="ExternalInput")
    wp_ext = nc.dram_tensor("w_proj", [128, CT, C], dt.float16, kind="ExternalInput")
    bp_ext = nc.dram_tensor("b_proj", [128, C // 128], dt.float32, kind="ExternalInput")
    y_ext = nc.dram_tensor("y", [C, N], dt.float16, kind="ExternalOutput")

    with (
        tile.TileContext(nc) as tc,
        tc.tile_pool(name="persist", bufs=1) as pp,
        tc.tile_pool(name="qkrot", bufs=3) as qkrot,
        tc.tile_pool(name="ps", bufs=2, space="PSUM") as ps,
        tc.tile_pool(name="att_sb", bufs=12) as att_sb,
        tc.tile_pool(name="att_small", bufs=3) as att_small,
        tc.tile_pool(name="y_sb", bufs=6) as y_pool,
    ):
        def ps_st(name):
            return ps.tile([128, N], dt.float32, tag="st", bufs=2, name=name)

        def ps_pov(name):
            return ps.tile([128, QHW], dt.float32, tag="pov", bufs=2, name=name)

        def ps_qkp(name):
            return ps.tile([128, QHW], dt.float32, tag="qkp", bufs=2, name=name)

        # ---- constants / biases ----
        ones_f32 = pp.tile([128, NH, 1], dt.float32, tag="ones_f32")
        nc.vector.memset(ones_f32[:], 1.0)
        bqk_sb = pp.tile([128, 2 * C // 128], dt.float32, tag="bqk")
        bv_sb = pp.tile([128, C], dt.float32, tag="bv")
        bp_sb = pp.tile([128, C // 128], dt.float32, tag="bp")

        dummy_sb = pp.tile([128, QHW], dt.float16, tag="dummy_sb")
        nc.vector.memset(dummy_sb[:].bitcast(dt.float32), 0.0)

        # ---- input loads: one priority-ordered queue for the big tensors
        # (descriptors drain in order). Every copy is contiguous because the
        # host pre-tiled the arrays. Biases ride a second queue (tiny).
        xt_all = pp.tile([128, CT, N], dt.float16, tag="xt_all")
        wq_r0_t = pp.tile([128, CT, 128], dt.float16, tag="wq_r0")
        wq_r6_t = pp.tile([128, CT, 128], dt.float16, tag="wq_r6")
        wq_v_t = pp.tile([128, CT, C], dt.float16, tag="wq_v")
        wq_q_t = pp.tile([128, CT, 640], dt.float16, tag="wq_q")
        wq_k_t = pp.tile([128, CT, 640], dt.float16, tag="wq_k")
        wp_all = pp.tile([128, CT, C], dt.float16, tag="wp_all")

        # x^T lands per-ct so the first qk-chain matmuls start as soon as the
        # first 256KB arrives (the DMA engines ramp slowly early on); V lands
        # in ct-pair blocks consumed in order by att0's V-chain fillers
        nc.sync.dma_start(out=wq_r0_t[:], in_=wq_r0_ext[:, :, :])
        nc.sync.dma_start(out=wq_r6_t[:], in_=wq_r6_ext[:, :, :])
        for i in range(CT):
            nc.sync.dma_start(out=xt_all[:, i, :], in_=xt_ext[:, i, :])
        for i in range(3):
            nc.sync.dma_start(
                out=wq_v_t[:, 2 * i:2 * i + 2, :], in_=wq_v_ext[:, 2 * i:2 * i + 2, :]
            )
        nc.sync.dma_start(out=wq_q_t[:], in_=wq_q_ext[:, :, :])
        nc.sync.dma_start(out=wq_k_t[:], in_=wq_k_ext[:, :, :])
        nc.sync.dma_start(out=wp_all[:], in_=wp_ext[:, :, :])
        nc.gpsimd.dma_start(out=bqk_sb[:], in_=bqk_ext[:, :])
        nc.gpsimd.dma_start(out=bv_sb[:], in_=bv_ext[0:1, :].to_broadcast((128, C)))
        nc.gpsimd.dma_start(out=bp_sb[:], in_=bp_ext[:, :])

        xt_sb = [xt_all[:, i, :] for i in range(CT)]
        wp_sb = [wp_all[:, i, :] for i in range(CT)]

        def wq_stat(d3, ct):
            """Stationary [128, 128] for Q/K d3-tile from the reordered regions."""
            if d3 == 0:
                return wq_r0_t[:, ct, :]
            if d3 == 6:
                return wq_r6_t[:, ct, :]
            if d3 < 6:
                return wq_q_t[:, ct, 128 * (d3 - 1):128 * d3]
            return wq_k_t[:, ct, 128 * (d3 - 7):128 * (d3 - 6)]

        # attn_out^T: 6 persistent tiles of [128, N]
        aT = [pp.tile([128, N], dt.float16, name=f"aT{i}", tag=f"aT{i}") for i in range(CT)]
        # V per token-tile [128, NH, 128] fp16: col 0 = ones (softmax
        # denominator row), cols 1:64 zero, cols 64:128 = V for that head.
        v_sb = [pp.tile([128, NH, 128], dt.float16, name=f"v{i}", tag=f"v{i}") for i in range(TT)]
        # proj ct0..4 partials (+ bias), fp16
        yacc = [pp.tile([128, N], dt.float16, name=f"yacc{i}", tag=f"yacc{i}")
                for i in range(CT)]

        # rotating Q^T/K^T tiles, keyed by d3-tile index
        qkT = {}

        def qk_half_thunks(d3, qh):
            """One qh-half of a Q^T/K^T projection chain: 6 matmuls into a
            [128,512] psum + bias evac into the rotating qkT tile. K tiles
            (d3 >= 6) are written as TWO zero-padded variants (even head in
            rows 0:64 / zeros, odd head in rows 64:128 / zeros) so the score
            matmuls run with K=128 contraction against the packed Q — keeps
            the PE in 128x128 tiling mode for the whole kernel."""
            thunks = []
            if qh == 0:
                thunks.extend(qk_make_tiles(d3))

            pss = ps_qkp(f"ps_qk{d3}_{qh}")

            def mk_mm(ct):
                def run():
                    nc.tensor.matmul(
                        out=pss[:],
                        lhsT=wq_stat(d3, ct),
                        rhs=xt_sb[ct][:, QHW * qh:QHW * (qh + 1)],
                        start=(ct == 0), stop=(ct == CT - 1),
                    )
                return run

            for ct in range(CT):
                thunks.append(mk_mm(ct))
            thunks.append(lambda: qk_bias(d3, qh, pss))
            return thunks

        def qk_make_tiles(d3):
            """Create the rotating qkT tile(s) for d3; returns setup thunks."""
            if d3 < 6:
                qkT[d3] = qkrot.tile([128, N], dt.float16, tag="qv", name=f"qkT{d3}")
                return []
            kz_e = qkrot.tile([128, N], dt.float16, tag="kve", name=f"kzE{d3}")
            kz_o = qkrot.tile([128, N], dt.float16, tag="kvo", name=f"kzO{d3}")
            qkT[d3] = (kz_e, kz_o)

            def zero():
                nc.vector.memset(kz_e[64:128, :].bitcast(dt.float32), 0.0)
                nc.vector.memset(kz_o[0:64, :].bitcast(dt.float32), 0.0)
            return [zero]

        def qk_bias(d3, qh, pss):
            if d3 < 6:
                nc.vector.tensor_scalar(
                    out=qkT[d3][:, QHW * qh:QHW * (qh + 1)], in0=pss[:],
                    scalar1=bqk_sb[:, d3:d3 + 1], scalar2=None, op0=ALU.add,
                )
            else:
                kz_e, kz_o = qkT[d3]
                nc.vector.tensor_scalar(
                    out=kz_e[0:64, QHW * qh:QHW * (qh + 1)],
                    in0=pss[0:64, :],
                    scalar1=bqk_sb[0:64, d3:d3 + 1], scalar2=None, op0=ALU.add,
                )
                nc.vector.tensor_scalar(
                    out=kz_o[64:128, QHW * qh:QHW * (qh + 1)],
                    in0=pss[64:128, :],
                    scalar1=bqk_sb[64:128, d3:d3 + 1], scalar2=None, op0=ALU.add,
                )

        def qk_chain_thunks(d3):
            return qk_half_thunks(d3, 0) + qk_half_thunks(d3, 1)

        def att_head(h, fillers=(), last_slot_fillers=()):
            q_tile = qkT[h // 2]
            kz = qkT[6 + h // 2][h % 2]
            po = 64 * (h % 2)
            fillers = list(fillers)
            # Software-pipelined head: per kt-slot emit fillers, then
            # PV(kt-2), then ST(kt)+exp(kt). ST waits on exp(kt-2)'s PSUM
            # bank; emitting the independent work first keeps the in-order
            # PE queue busy through that wait.
            PIPE = 2
            ess = []
            povs = [ps_pov(f"pov{h}_{qh}") for qh in range(NQH)]
            fi = 0
            # 2 kt per slot: batching two kt of STs (and two kt of PVs) per
            # slot halves the psum-ring handoffs.
            NSLOT = TT // 2 + PIPE

            def pv_one(kt, qh):
                nc.tensor.matmul(
                    out=povs[qh][:],
                    lhsT=v_sb[kt][:, h, :],
                    rhs=ess[kt][:, QHW * qh:QHW * (qh + 1)],
                    start=(kt == 0), stop=(kt == TT - 1),
                )

            def pv_pair(kt):
                for qh in range(NQH):
                    pv_one(kt, qh)

            def norm_one(qh):
                # normalize rows 64:128 by reciprocal of denominator row 0:
                # DVE reciprocal, GpSimd broadcast, DVE multiply (GpSimd
                # cannot read PSUM)
                r_sb = att_small.tile([1, QHW], dt.float32, tag="r")
                nc.vector.reciprocal_approx_fast(out=r_sb[:], in_=povs[qh][0:1, :])
                rb_sb = att_small.tile([HD, QHW], dt.float32, tag="rb")
                nc.gpsimd.partition_broadcast(rb_sb[:], r_sb[:])
                nc.vector.tensor_tensor(
                    out=aT[h // 2][po:po + HD, QHW * qh:QHW * (qh + 1)],
                    in0=povs[qh][64:128, :],
                    in1=rb_sb[:],
                    op=ALU.mult,
                )

            for sl in range(NSLOT):
                # fillers first: anything a PV may consume (e.g. att0's
                # V tiles) must be emitted before the PV that reads it
                want = min(len(fillers), -(-(sl * len(fillers)) // (NSLOT - 2)))
                while fi < want:
                    fillers[fi]()
                    fi += 1
                if sl >= PIPE:
                    k0 = 2 * (sl - PIPE)
                    if sl == NSLOT - 1:
                        # eager norm: finish qh0's accumulation (any kt order
                        # is fine — it's a sum) and start its normalize chain
                        # while qh1's PVs (and any held-back fillers) stream
                        pv_one(k0, 0)
                        pv_one(k0 + 1, 0)
                        norm_one(0)
                        for th in last_slot_fillers:
                            th()
                        pv_one(k0, 1)
                        pv_one(k0 + 1, 1)
                        norm_one(1)
                    else:
                        pv_pair(k0)
                        pv_pair(k0 + 1)
                if sl < TT // 2:
                    pair = []
                    for kt in (2 * sl, 2 * sl + 1):
                        pss = ps_st(f"pss{h}_{kt}")
                        for qh in range(NQH):
                            nc.tensor.matmul(
                                out=pss[:, QHW * qh:QHW * (qh + 1)],
                                lhsT=kz[:, 128 * kt:128 * (kt + 1)],
                                rhs=q_tile[:, QHW * qh:QHW * (qh + 1)],
                                start=True, stop=True,
                            )
                        pair.append(pss)
                    for kt in (2 * sl, 2 * sl + 1):
                        es = att_sb.tile([128, N], dt.float16, tag="es", name=f"es{h}_{kt}")
                        nc.scalar.activation(
                            out=es[:], in_=pair[kt - 2 * sl][:], func=AF.Exp, scale=SCALE
                        )
                        ess.append(es)
            while fi < len(fillers):
                fillers[fi]()
                fi += 1

        def att_head_qh(h, qh, fillers=(), filler_start=0, post_loop_fillers=(),
                        filler_span=None):
            """One qh-phase of a head (used for the final head so the proj
            tail for the finished half can stream as the other half's
            fillers). STs write a [kt_even | kt_odd] pair tile for this qh
            only, so exp stays one [128, N] ACT per slot — same ScalarE cost
            as the full-head path. `filler_start` delays filler emission to
            slot >= filler_start so fillers gated on this head's earlier
            phase don't head-of-line block the PE queue."""
            q_tile = qkT[h // 2]
            kz = qkT[6 + h // 2][h % 2]
            po = 64 * (h % 2)
            fillers = list(fillers)
            PIPE = 2
            NSLOT = TT // 2 + PIPE
            pov = ps_pov(f"pov{h}q{qh}")
            espairs = []
            fi = 0

            def pv_one(kt):
                nc.tensor.matmul(
                    out=pov[:],
                    lhsT=v_sb[kt][:, h, :],
                    rhs=espairs[kt // 2][:, QHW * (kt % 2):QHW * (kt % 2 + 1)],
                    start=(kt == 0), stop=(kt == TT - 1),
                )

            for sl in range(NSLOT):
                eff = max(0, sl - filler_start)
                span = (filler_span if filler_span is not None
                        else NSLOT - 2 - filler_start)
                want = min(len(fillers), -(-(eff * len(fillers)) // span))
                while fi < want:
                    fillers[fi]()
                    fi += 1
                if sl >= PIPE:
                    k0 = 2 * (sl - PIPE)
                    pv_one(k0)
                    pv_one(k0 + 1)
                if sl < TT // 2:
                    pss = ps_st(f"pss{h}q{qh}_{sl}")
                    for kt in (2 * sl, 2 * sl + 1):
                        nc.tensor.matmul(
                            out=pss[:, QHW * (kt % 2):QHW * (kt % 2 + 1)],
                            lhsT=kz[:, 128 * kt:128 * (kt + 1)],
                            rhs=q_tile[:, QHW * qh:QHW * (qh + 1)],
                            start=True, stop=True,
                        )
                    es = att_sb.tile([128, N], dt.float16, tag="es",
                                     name=f"es{h}q{qh}_{sl}")
                    nc.scalar.activation(out=es[:], in_=pss[:], func=AF.Exp, scale=SCALE)
                    espairs.append(es)
            while fi < len(fillers):
                fillers[fi]()
                fi += 1
            # post-loop fillers: PE-only work (no DVE coupling) emitted just
            # before the normalize so it streams under the chain's latency
            for th in post_loop_fillers:
                th()
            # this phase's normalize: the next phase's (or the tail's) PE
            # stream covers the reciprocal->broadcast->multiply latency
            r_sb = att_small.tile([1, QHW], dt.float32, tag="r")
            nc.vector.reciprocal_approx_fast(out=r_sb[:], in_=pov[0:1, :])
            rb_sb = att_small.tile([HD, QHW], dt.float32, tag="rb")
            nc.gpsimd.partition_broadcast(rb_sb[:], r_sb[:])
            nc.vector.tensor_tensor(
                out=aT[h // 2][po:po + HD, QHW * qh:QHW * (qh + 1)],
                in0=pov[64:128, :],
                in1=rb_sb[:],
                op=ALU.mult,
            )

        # HAM warm-up: dummy matmuls with no input dependencies, all into ONE
        # psum tile — same-engine WAW ordering is implicit, so they run
        # back-to-back with no ring-semaphore waits
        pwarm = ps_qkp("pwarm")
        for i in range(N_WARMUP_MM):
            nc.tensor.matmul(
                out=pwarm[:], lhsT=dummy_sb[:, 0:128], rhs=dummy_sb[:],
                start=True, stop=True, skip_group_check=True,
            )

        # prologue chains for head-pair 0, ct-grouped: all four psum chains
        # run their ct 0..2 matmuls as soon as x^T's first half lands, while
        # the second half is still in flight
        for th in qk_make_tiles(0) + qk_make_tiles(6):
            th()
        pro_ps = {
            (0, 0): ps_qkp("ps_pro0_0"), (0, 1): ps_qkp("ps_pro0_1"),
            (6, 0): ps_pov("ps_pro6_0"), (6, 1): ps_pov("ps_pro6_1"),
        }
        for ct in range(CT):
            for d3 in (0, 6):
                for qh in range(NQH):
                    nc.tensor.matmul(
                        out=pro_ps[(d3, qh)][:],
                        lhsT=wq_stat(d3, ct),
                        rhs=xt_sb[ct][:, QHW * qh:QHW * (qh + 1)],
                        start=(ct == 0), stop=(ct == CT - 1),
                    )
        for d3 in (0, 6):
            for qh in range(NQH):
                qk_bias(d3, qh, pro_ps[(d3, qh)])

        # V part of the qkv projection, as thunk lists. Split per 512/256
        # column half so each psum lives in one [128,512] ring slot.
        def v_chain_thunks(tt):
            thunks = []

            def zero():
                nc.vector.memset(v_sb[tt][:].bitcast(dt.float32), 0.0)
            thunks.append(zero)

            psA = ps_qkp(f"ps_v{tt}_a")
            psB = ps_qkp(f"ps_v{tt}_b")

            def mk_mm(pst, w, c0, ct):
                def run():
                    nc.tensor.matmul(
                        out=pst[:, 0:w],
                        lhsT=xt_sb[ct][:, 128 * tt:128 * (tt + 1)],
                        rhs=wq_v_t[:, ct, c0:c0 + w],
                        start=(ct == 0), stop=(ct == CT - 1),
                    )
                return run

            for ct in range(CT):
                thunks.append(mk_mm(psA, 512, 0, ct))

            def finishA():
                nc.vector.tensor_tensor(
                    out=v_sb[tt][:, 0:8, 64:128],
                    in0=psA[:].rearrange("p (h d) -> p h d", h=8),
                    in1=bv_sb[:, 0:512].rearrange("p (h d) -> p h d", h=8),
                    op=ALU.add,
                )
            thunks.append(finishA)

            for ct in range(CT):
                thunks.append(mk_mm(psB, 256, 512, ct))

            def finishB():
                nc.vector.tensor_tensor(
                    out=v_sb[tt][:, 8:12, 64:128],
                    in0=psB[:, 0:256].rearrange("p (h d) -> p h d", h=4),
                    in1=bv_sb[:, 512:768].rearrange("p (h d) -> p h d", h=4),
                    op=ALU.add,
                )
                nc.vector.tensor_copy(out=v_sb[tt][:, :, 0:1], in_=ones_f32[:])
            thunks.append(finishB)
            return thunks

        # proj partials: per co, the ct 0..4 partial (heads 0..9's channels)
        # runs as one thunk per qh-half during heads 10/11 — 5 matmuls into a
        # [128,512] ring slot, then a bias-folding evac to fp16 yacc.
        def proj_partial_thunks(co):
            thunks = []
            for qh in range(NQH):
                def run(qh=qh):
                    psq = ps_qkp(f"ps_yp{co}_{qh}")
                    for ct in range(CT - 1):
                        nc.tensor.matmul(
                            out=psq[:],
                            lhsT=wp_sb[ct][:, 128 * co:128 * (co + 1)],
                            rhs=aT[ct][:, QHW * qh:QHW * (qh + 1)],
                            start=(ct == 0), stop=(ct == CT - 2),
                        )
                    nc.vector.tensor_scalar(
                        out=yacc[co][:, QHW * qh:QHW * (qh + 1)], in0=psq[:],
                        scalar1=bp_sb[:, co:co + 1], scalar2=None, op0=ALU.add,
                    )
                thunks.append(run)
            return thunks

        # attention heads 0..11 with remaining qkv work sprinkled into
        # each head's PE stream: att0 carries ALL V tiles; heads 1..7 carry
        # the Q/K chains for pairs 1..4 plus pair 5's first halves pushed to
        # heads 8/9 as qh-split halves so head 10's operands are ready a full
        # head early; heads 10/11 carry the proj ct0..4 partials.
        filler_plan = {
            0: lambda: [t for tt in range(TT) for t in v_chain_thunks(tt)],
            1: lambda: qk_chain_thunks(1) + qk_chain_thunks(7),
            2: lambda: qk_chain_thunks(2),
            3: lambda: qk_chain_thunks(8),
            4: lambda: qk_chain_thunks(3),
            5: lambda: qk_chain_thunks(9),
            6: lambda: qk_chain_thunks(4),
            7: lambda: qk_chain_thunks(10),
            8: lambda: qk_half_thunks(5, 0) + qk_half_thunks(11, 0),
            9: lambda: qk_half_thunks(5, 1) + qk_half_thunks(11, 1),
            10: lambda: [t for co in (0, 1, 2) for t in proj_partial_thunks(co)],
        }

        # ---- proj tail rungs: ct=5 contribution for one (co, qh) plus the
        # yacc add and the y DMA. qh0 rungs stream as fillers of head 11's
        # second attention phase; qh1 rungs drain after it. Evac engines
        # alternate (DVE psum-add vs ScalarE copy + DVE fp16 2x add) so the
        # final ladder isn't serialized on one engine.
        def tail_rung(co, qh):
            def run():
                pst = ps_qkp(f"ps_y{co}_{qh}")
                nc.tensor.matmul(
                    out=pst[:],
                    lhsT=wp_sb[CT - 1][:, 128 * co:128 * (co + 1)],
                    rhs=aT[CT - 1][:, QHW * qh:QHW * (qh + 1)],
                    start=True, stop=True,
                )
                y_half = y_pool.tile([128, QHW], dt.float16, tag="y")
                nc.vector.tensor_tensor(
                    out=y_half[:], in0=pst[:],
                    in1=yacc[co][:, QHW * qh:QHW * (qh + 1)], op=ALU.add,
                )
                nc.sync.dma_start(
                    out=y_ext[128 * co:128 * (co + 1), QHW * qh:QHW * (qh + 1)],
                    in_=y_half[:],
                )
            return run

        def tail_wide_qh1():
            # final drain: by now the score double-buffer banks are free, so
            # all six ct=5 matmuls get their own psum bank and run
            # back-to-back; evacs alternate DVE-direct / ScalarE-copy+DVE so
            # the ladder drains on two engines; all y DMAs issue on sync.
            psts = []
            for pair in range(2):
                stw = ps_st(f"ps_yw{pair}")
                psts += [stw[:, 0:QHW], stw[:, QHW:N]]
            psts += [ps_qkp("ps_yw4"), ps_qkp("ps_yw5")]
            # dummy matmuls stream while the final normalize chain
            # (reciprocal->broadcast->multiply) completes, so the PE stays at
            # full p-state and the real tail matmuls don't pay the
            # half-clock restart penalty (the first real matmul overwrites
            # the dummies' bank with start=True)
            for _ in range(8):
                nc.tensor.matmul(
                    out=psts[0][:], lhsT=dummy_sb[:, 0:128], rhs=dummy_sb[:],
                    start=True, stop=True, skip_group_check=True,
                )
            for co in range(CT):
                nc.tensor.matmul(
                    out=psts[co][:],
                    lhsT=wp_sb[CT - 1][:, 128 * co:128 * (co + 1)],
                    rhs=aT[CT - 1][:, QHW:N],
                    start=True, stop=True,
                )
            for co in range(CT):
                # all-DVE evacs: back-to-back ~690ns adds beat the
                # ScalarE-copy chain (copies observed starting ~1us late)
                y_half = y_pool.tile([128, QHW], dt.float16, tag="y")
                nc.vector.tensor_tensor(
                    out=y_half[:], in0=psts[co][:],
                    in1=yacc[co][:, QHW:N], op=ALU.add,
                )
                # alternate issue queues so six ~0.6us DMA issues don't
                # serialize on one engine at the very end
                (nc.sync if co % 2 == 0 else nc.gpsimd).dma_start(
                    out=y_ext[128 * co:128 * (co + 1), QHW:N],
                    in_=y_half[:],
                )

        for h in range(NH - 1):
            att_head(h, filler_plan[h]() if h in filler_plan else ())
        # head 11 runs as two sequential qh-phases: phase 0 carries the
        # remaining proj partials; phase 1 carries the qh0 proj-tail rungs
        # (gated on phase 0's norm — filler_start keeps them from
        # head-of-line blocking phase 1's first STs)
        # head 11 phase 0 carries co3/co4 partials and co5's qh0 half; co5's
        # qh1-half matmuls stream under phase 1's normalize chain (its DVE
        # evac is deferred until after the norm so it can't delay the
        # reciprocal->multiply chain), then the wide tail drains.
        co5q1_ps = []

        def co5q1_mms():
            psq = ps_qkp("ps_yp5_1")
            co5q1_ps.append(psq)
            for ct in range(CT - 1):
                nc.tensor.matmul(
                    out=psq[:],
                    lhsT=wp_sb[ct][:, 128 * 5:128 * 6],
                    rhs=aT[ct][:, QHW:N],
                    start=(ct == 0), stop=(ct == CT - 2),
                )

        att_head_qh(NH - 1, 0,
                    [t for co in (3, 4) for t in proj_partial_thunks(co)]
                    + proj_partial_thunks(5)[:1],
                    filler_span=5)
        att_head_qh(NH - 1, 1, [tail_rung(co, 0) for co in range(CT)],
                    filler_start=2, post_loop_fillers=[co5q1_mms])
        nc.vector.tensor_scalar(
            out=yacc[5][:, QHW:N], in0=co5q1_ps[0][:],
            scalar1=bp_sb[:, 5:6], scalar2=None, op0=ALU.add,
        )
        tail_wide_qh1()

    nc.compile()
    return nc


_NC_CACHE = {}


def kernel(x, w_qkv, b_qkv, w_proj, b_proj, _trace=False):
    x = np.asarray(x, dtype=np.float32)
    w_qkv = np.asarray(w_qkv, dtype=np.float32)
    b_qkv = np.asarray(b_qkv, dtype=np.float32)
    w_proj = np.asarray(w_proj, dtype=np.float32)
    b_proj = np.asarray(b_proj, dtype=np.float32)

    if "nc" not in _NC_CACHE:
        _NC_CACHE["nc"] = _build_nc()
    nc = _NC_CACHE["nc"]

    # host-side prep (pure layout, no arithmetic). Big tensors are shipped
    # pre-tiled [128, CT, w] so every device DMA is a contiguous stream.
    def tiled(a2d):
        # [CT*128, w] -> [128, CT, w]
        ctn = a2d.shape[0] // 128
        return np.ascontiguousarray(
            a2d.reshape(ctn, 128, a2d.shape[1]).transpose(1, 0, 2)
        )

    # b_qkt: Q/K bias columns laid out per d3-tile: [128, 12]
    b_qkt = np.ascontiguousarray(b_qkv[:2 * C].reshape(2 * C // 128, 128).T)
    w_qkv_h = w_qkv.astype(np.float16)
    wq_r0 = tiled(w_qkv_h[:, 0:128])
    wq_r6 = tiled(w_qkv_h[:, 768:896])
    wq_v = tiled(w_qkv_h[:, 1536:2304])
    wq_q = tiled(w_qkv_h[:, 128:768])
    wq_k = tiled(w_qkv_h[:, 896:1536])
    w_proj_t = tiled(w_proj.astype(np.float16))
    b_v = np.ascontiguousarray(b_qkv[2 * C:].reshape(1, C))
    b_p = np.ascontiguousarray(b_proj.reshape(C // 128, 128).T)

    core_ids = list(range(B))
    in_maps = []
    for b in range(B):
        xt = tiled(x[b].reshape(N, C).T.astype(np.float16))  # [128, 6, N]
        in_maps.append({
            "xt": xt,
            "wq_r0": wq_r0,
            "wq_r6": wq_r6,
            "wq_v": wq_v,
            "wq_q": wq_q,
            "wq_k": wq_k,
            "b_qkt": b_qkt,
            "b_v": b_v,
            "w_proj": w_proj_t,
            "b_proj": b_p,
        })

    res = run_bass_kernel_spmd(nc, in_maps, core_ids, trace=_trace)
    if _trace:
        _NC_CACHE["last_result"] = res

    out = np.empty((B, 32, 32, C), dtype=np.float32)
    for b in range(B):
        out[b] = res.results[b]["y"].T.reshape(32, 32, C)
    return out
